# revision 14
# baseline (speedup 1.0000x reference)
"""TAGConv-style GNN encoder (degree-normalized edge aggregation + linear +
L2 row-normalize) on 8 Trainium2 NeuronCores.

Strategy (dst-sharded, fully data-parallel — no collectives):
  - Nodes sharded by destination: core c owns dst rows [c*NPC, (c+1)*NPC).
  - Host-side metadata: edges (with multiplicity — no dedup) are laid out
    into 128-edge tiles grouped by (256-wide dst window, src-chunk of 25000
    rows). The tile schedule is shared across cores (padded to the
    per-region max) so one SPMD program serves all 8.
  - Gather: the h table is pre-scaled by rsqrt(deg_src) on host (bf16).
    Per region, one big gpsimd dma_gather in PREPARE_ONLY mode writes SWDGE
    descriptors; trigger_dma fires them. 4 SWDGE queues + a 4096-descriptor
    ring let descriptor generation overlap the DMA transfers, so the DMA
    engines (not the gpsimd ucode) are the limiter.
  - Scatter: DVE tensor_scalar builds per-tile one-hot segment matrices
    oh[slot, dstoff] = (iota == offs[slot]) * rsqrt(deg_dst[slot]) in one
    4x-mode instruction per tile; TensorE matmul G.T @ oh accumulates
    segment sums in PSUM across tiles (has_written semantics).
  - Tail: out^T = W1.T @ h^T + W2.T @ agg^T (+bias), L2-normalize columns
    via ones-matmul partition reduction + scalar-engine Rsqrt. h^T comes
    pre-transposed from the host. Output is written transposed
    [128, NPC_padded]; the host transposes/concatenates shards.
"""
import numpy as np
import ml_dtypes

import concourse.bass as bass
import concourse.tile as tile
from concourse import mybir, bacc
from concourse.bass_utils import run_bass_kernel_spmd

F32 = mybir.dt.float32
BF16 = mybir.dt.bfloat16
I32 = mybir.dt.int32
I16 = mybir.dt.int16


def _patched_drain_and_barrier(self, tick_clock, wait_clock):
    """Tile's kernel-tail Drain carries one sync-wait per outstanding
    semaphore; the walrus build in this container can't encode more than one
    wait on one instruction. Emit each wait as its own wait_ge instead."""
    nc = self.nc
    probe = nc.sync.nop(nofuse=True)
    wait_clock.add_sem_waits(probe.ins, tile.ScopedClock({None: tick_clock.global_clock}))
    si = probe.ins.sync_info
    waits = list(si.on_wait) if si is not None else []
    if len(waits) > 1:
        si.on_wait.clear()
        sem_by_num = {h.num: h for h in self.sems.allocated().values()}
        for w in waits:
            nc.sync.wait_ge(sem_by_num[w.id], w.wait_value)
    nc.sync.drain()
    nc.all_engine_barrier()
    popped = nc._tile_sem_poison_stack.pop()
    assert popped is self._sem_poison
    nc.clear_and_free_semaphores(list(self.sems.allocated().values()))
    nc.all_engine_barrier()


tile.TileContext._drain_and_barrier = _patched_drain_and_barrier

# this walrus build encodes at most this many sync waits on one instruction
MAX_WAITS = 1


def _split_excess_waits(nc, max_waits=MAX_WAITS):
    """Hoist sync waits beyond the per-instruction ISA budget onto NoOps
    inserted just before the instruction (same engine queue, so ordering
    semantics are identical). Must run AFTER Bacc.compile (its nop-fusion
    passes would re-merge the waits)."""
    for f in nc.m.functions:
        for b in f.blocks:
            ins_list = b.instructions
            out_list = []
            changed = False
            for ins in ins_list:
                si = ins.sync_info
                waits = list(si.on_wait) if si is not None else []
                if len(waits) > max_waits:
                    excess, keep = waits[:-max_waits], waits[-max_waits:]
                    for j in range(0, len(excess), max_waits):
                        nop = mybir.InstNoOp(
                            name=nc.get_next_instruction_name(), ins=[], outs=[])
                        nop.engine = ins.engine
                        nop.sync_info = mybir.SyncInfo(
                            on_wait=excess[j:j + max_waits], on_update=[])
                        out_list.append(nop)
                    ins.sync_info = mybir.SyncInfo(
                        on_wait=keep, on_update=list(si.on_update))
                    changed = True
                out_list.append(ins)
            if changed:
                b.instructions = out_list


# Problem constants (hardcoded: harness contract)
N_NODES = 100000
D = 128
HID = 128
CORES = 8

# Kernel tuning
WIN = 256         # dst window width = segment-matmul N
TILE = 128        # edge slots per tile (= matmul K)
BANK = 512        # PSUM bank width in f32 cols
CHUNK_WINS = 6    # windows per PSUM chunk (6*256 = 1536 cols = 3 banks)
GXMAX = 8         # max tiles per dma_gather
SCH = 4           # src chunks (int16 gather indices => table <= 32767 rows)
NQ = 4            # SWDGE queues
SCRATCH = 16384   # dynamic DMA scratch bytes/partition
PREP = True       # prepare_only + trigger_dma (False: blocking dma_gather)
OFFS_PAD = 512.0  # one-hot match value for pad slots (never matches iota)


def _preprocess(src, dst, n_nodes, npc, cores):
    """Host-side edge partitioning (integer index metadata only)."""
    assert n_nodes % SCH == 0
    cn = n_nodes // SCH
    assert cn < 32768, "src-chunk must fit int16 gather indices"
    src = np.asarray(src).astype(np.int64)
    dst = np.asarray(dst).astype(np.int64)
    E = len(src)
    deg = np.bincount(dst, minlength=n_nodes)
    nrm = 1.0 / np.sqrt(np.maximum(deg, 1).astype(np.float64))

    core_of = dst // npc
    ldst = dst - core_of * npc
    win = ldst // WIN
    kch = src // cn
    n_wins = (npc + WIN - 1) // WIN
    n_regions = n_wins * SCH
    region = win * SCH + kch  # (w, k) region id within a core

    # program emission order of regions: psum-chunk major, then k, then w
    order_regions = []
    for p0 in range(0, n_wins, CHUNK_WINS):
        p1 = min(n_wins, p0 + CHUNK_WINS)
        for k in range(SCH):
            for w in range(p0, p1):
                order_regions.append(w * SCH + k)
    order_regions = np.array(order_regions, np.int64)
    region_pos = np.empty(n_regions, np.int64)
    region_pos[order_regions] = np.arange(n_regions)

    # per-core edge counts per region; shared tile schedule = per-region max
    cnt = np.zeros((cores, n_regions), np.int64)
    for c in range(cores):
        m = core_of == c
        cnt[c] = np.bincount(region[m], minlength=n_regions)
    tiles_r = -(-cnt.max(axis=0) // TILE)  # ceil
    # every window must write its PSUM cols at least once
    win_tiles = tiles_r.reshape(n_wins, SCH).sum(axis=1)
    for w in np.nonzero(win_tiles == 0)[0]:
        tiles_r[w * SCH] = 1

    T_sorted = tiles_r[order_regions]
    tile_base_sorted = np.zeros(n_regions + 1, np.int64)
    np.cumsum(T_sorted, out=tile_base_sorted[1:])
    n_tiles = int(tile_base_sorted[-1])
    n_slots = n_tiles * TILE
    slot_base_pos = tile_base_sorted[:-1] * TILE  # by emission pos

    # slot assignment: sort edges by (core, region emission pos, src)
    rpos = region_pos[region]
    glob = np.lexsort((src, rpos, core_of))
    cs, rs = core_of[glob], rpos[glob]
    runkey = cs * n_regions + rs
    starts = np.searchsorted(runkey, np.arange(cores * n_regions))
    rank = np.arange(E) - starts[runkey]
    slot = slot_base_pos[rs] + rank

    core_bounds = np.searchsorted(cs, np.arange(cores + 1))

    per_core = []
    for c in range(cores):
        s, e = core_bounds[c], core_bounds[c + 1]
        sl = slot[s:e]
        g = glob[s:e]
        gidx = np.zeros(n_slots, np.int16)        # pads gather row 0 of chunk
        offs = np.full(n_slots, OFFS_PAD, np.float32)
        nrmd = np.zeros(n_slots, np.float32)
        gidx[sl] = (src[g] - kch[g] * cn).astype(np.int16)
        offs[sl] = (ldst[g] - win[g] * WIN).astype(np.float32)
        nrmd[sl] = nrm[dst[g]].astype(np.float32)

        # [n_slots] -> [128, n_tiles]: slot j of tile t at [j, t]
        def t_(a, dt):
            return np.ascontiguousarray(a.reshape(n_tiles, TILE).T.astype(dt))

        # int16 idx wrap for dma_gather: within-instruction idx i at
        # [i % 16, i // 16], replicated across the 8 16-partition groups.
        # Instruction = run of whole tiles, so per-tile 8-col blocks suffice.
        a = gidx.reshape(n_tiles, 8, 16)          # [t, i//16, i%16]
        wrapped = np.transpose(a, (2, 0, 1)).reshape(16, n_tiles * 8)
        gidx16 = np.ascontiguousarray(np.tile(wrapped, (8, 1)))  # [128, 8*NT]

        per_core.append(dict(
            gidx16=gidx16,
            offs=t_(offs, np.float32),
            nrmd=t_(nrmd, np.float32),
        ))

    # gather batches: consecutive same-k regions packed up to GXMAX tiles,
    # never crossing a psum-chunk boundary
    groups = []  # (k, t_start, t_end)
    n_chunks = -(-n_wins // CHUNK_WINS)
    pos = 0
    t_acc = 0
    for p0 in range(0, n_wins, CHUNK_WINS):
        p1 = min(n_wins, p0 + CHUNK_WINS)
        for k in range(SCH):
            run = []  # tile counts of this (chunk, k) run of regions
            for w in range(p0, p1):
                run.append(int(tiles_r[w * SCH + k]))
            total = sum(run)
            # split [t_acc, t_acc+total) into <=GXMAX batches
            a = 0
            while a < total:
                b = min(total, a + GXMAX)
                groups.append((k, t_acc + a, t_acc + b))
                a = b
            t_acc += total
            pos += len(run)
    assert t_acc == n_tiles

    # window of each tile (for matmul column/bank mapping)
    win_of_tile = np.empty(n_tiles, np.int64)
    t = 0
    for p0 in range(0, n_wins, CHUNK_WINS):
        p1 = min(n_wins, p0 + CHUNK_WINS)
        for k in range(SCH):
            for w in range(p0, p1):
                nt = int(tiles_r[w * SCH + k])
                win_of_tile[t:t + nt] = w
                t += nt

    return dict(
        groups=groups,
        n_wins=n_wins,
        n_tiles=n_tiles,
        win_of_tile=win_of_tile,
        nrm=nrm,
        per_core=per_core,
    )


def _build_program(sched, n_nodes, npc, split_waits=True):
    """Build the single SPMD Bass/Tile program (identical for all cores)."""
    n_wins = sched["n_wins"]
    n_tiles = sched["n_tiles"]
    win_of_tile = sched["win_of_tile"]
    groups = sched["groups"]
    cn = n_nodes // SCH
    padn = n_wins * WIN            # padded local dst count (cols of out^T)
    n_chunks = -(-n_wins // CHUNK_WINS)

    nc = bacc.Bacc("TRN2", target_bir_lowering=False,
                   num_swdge_queues=NQ, dynamic_dma_scratch_size=SCRATCH)
    hb = nc.declare_dram_parameter("hb", [n_nodes, D], BF16, isOutput=False)
    hTd = nc.declare_dram_parameter("hTd", [D, padn], BF16, isOutput=False)
    gidx_p = nc.declare_dram_parameter("gidx16", [TILE, 8 * n_tiles], I16, isOutput=False)
    offs_p = nc.declare_dram_parameter("offs", [TILE, n_tiles], F32, isOutput=False)
    nrmd_p = nc.declare_dram_parameter("nrmd", [TILE, n_tiles], F32, isOutput=False)
    wt_p = nc.declare_dram_parameter("wt", [2 * D, HID], BF16, isOutput=False)
    bias_p = nc.declare_dram_parameter("bias_c", [HID, 1], F32, isOutput=False)
    out_p = nc.declare_dram_parameter("out", [HID, padn], F32, isOutput=True)

    # tiles grouped per psum chunk
    chunk_of_tile = win_of_tile // CHUNK_WINS

    with tile.TileContext(nc) as tc:
        with (
            tc.tile_pool(name="const", bufs=1) as const,
            tc.tile_pool(name="g", bufs=3) as gpool,
            tc.tile_pool(name="oh", bufs=3) as ohpool,
            tc.tile_pool(name="ht", bufs=2) as htpool,
            tc.tile_pool(name="at", bufs=2) as atpool,
            tc.tile_pool(name="y", bufs=6) as ypool,
            tc.tile_pool(name="aggps", bufs=1, space="PSUM") as agg_ps,
            tc.tile_pool(name="scrps", bufs=2, space="PSUM") as scr_ps,
        ):
            # ---- constants / metadata ----
            gidx_sb = const.tile([TILE, 8 * n_tiles], I16)
            nc.sync.dma_start(gidx_sb[:], gidx_p[:])
            offs_sb = const.tile([TILE, n_tiles], F32)
            nc.sync.dma_start(offs_sb[:], offs_p[:])
            nrmd_sb = const.tile([TILE, n_tiles], F32)
            nc.sync.dma_start(nrmd_sb[:], nrmd_p[:])

            w1_sb = const.tile([D, HID], BF16)
            nc.sync.dma_start(w1_sb[:], wt_p[0:D, :])
            w2_sb = const.tile([D, HID], BF16)
            nc.sync.dma_start(w2_sb[:], wt_p[D:2 * D, :])
            bias_sb = const.tile([HID, 1], F32)
            nc.sync.dma_start(bias_sb[:], bias_p[:])
            ones_sb = const.tile([128, 128], BF16)
            nc.vector.memset(ones_sb[:], 1.0)
            iota_i = const.tile([128, WIN], I32)
            nc.gpsimd.iota(iota_i[:], pattern=[[1, WIN]], base=0, channel_multiplier=0)
            iota_b = const.tile([128, WIN], BF16)
            nc.vector.tensor_copy(iota_b[:], iota_i[:])

            # shared num_idxs registers for dma_gather (one per distinct size)
            ni_regs = {}

            def ni_reg(n):
                if n not in ni_regs:
                    r = nc.gpsimd.alloc_register()
                    nc.gpsimd.reg_mov(r, n)
                    ni_regs[n] = r
                return ni_regs[n]

            dma_sems = [nc.alloc_semaphore(f"gdma{q}") for q in range(NQ)]

            # per-chunk batch lists
            batches_of_chunk = [[] for _ in range(n_chunks)]
            for gi, (k, ta, tb) in enumerate(groups):
                ch = int(chunk_of_tile[ta])
                assert int(chunk_of_tile[tb - 1]) == ch
                batches_of_chunk[ch].append((k, ta, tb))

            qrr = 0  # round-robin queue cursor
            qcount = [0] * NQ  # completed-DMA wait targets per queue

            # ---- main loop over dst chunks ----
            for ch in range(n_chunks):
                w0 = ch * CHUNK_WINS
                w1 = min(n_wins, w0 + CHUNK_WINS)
                cw = (w1 - w0) * WIN
                col0 = w0 * WIN

                tlist = [t for (k, ta, tb) in batches_of_chunk[ch]
                         for t in range(ta, tb)]
                # first/last program-order touch per psum bank in this chunk
                first_of_bank, last_of_bank = {}, {}
                for t in tlist:
                    bk = (int(win_of_tile[t]) - w0) * WIN // BANK
                    first_of_bank.setdefault(bk, t)
                    last_of_bank[bk] = t

                pagg = agg_ps.tile([128, CHUNK_WINS * WIN], F32, tag="pagg")

                for (k, ta, tb) in batches_of_chunk[ch]:
                    gt = tb - ta
                    G = gpool.tile([128, GXMAX, D], BF16, tag="G")
                    q = qrr % NQ
                    qrr += 1
                    qcount[q] += 1
                    gwait = 16 * qcount[q]
                    if PREP:
                        nc.gpsimd.dma_gather(
                            out_ap=G[:, :gt, :],
                            in_ap=hb[k * cn:(k + 1) * cn, :],
                            idxs_ap=gidx_sb[:, 8 * ta:8 * tb],
                            num_idxs=TILE * gt,
                            num_idxs_reg=ni_reg(TILE * gt),
                            elem_size=D,
                            prepare_only=True,
                            sem=dma_sems[q],
                            queue_num=q,
                        )
                        nc.gpsimd.trigger_dma(count=None, queue_num=q)
                    else:
                        nc.gpsimd.dma_gather(
                            out_ap=G[:, :gt, :],
                            in_ap=hb[k * cn:(k + 1) * cn, :],
                            idxs_ap=gidx_sb[:, 8 * ta:8 * tb],
                            num_idxs=TILE * gt,
                            num_idxs_reg=ni_reg(TILE * gt),
                            elem_size=D,
                            queue_num=q,
                        )

                    oh = ohpool.tile([128, GXMAX, WIN], BF16, tag="oh")
                    for x in range(gt):
                        t = ta + x
                        nc.vector.tensor_scalar(
                            out=oh[:, x, :],
                            in0=iota_b[:],
                            scalar1=offs_sb[:, t:t + 1],
                            scalar2=nrmd_sb[:, t:t + 1],
                            op0=mybir.AluOpType.is_equal,
                            op1=mybir.AluOpType.mult,
                        )
                    if PREP:
                        # Tile's DMASW lane sems are pre-bumped for SWDGE
                        # preps; enforce the gather-DMA RAW edge explicitly.
                        nc.tensor.wait_ge(dma_sems[q], gwait)
                    for x in range(gt):
                        t = ta + x
                        col = (int(win_of_tile[t]) - w0) * WIN
                        bk = col // BANK
                        nc.tensor.matmul(
                            pagg[:, col:col + WIN],
                            lhsT=G[:, x, :],
                            rhs=oh[:, x, :],
                            start=(first_of_bank[bk] == t),
                            stop=(last_of_bank[bk] == t),
                            skip_group_check=True,
                        )

                # evacuate agg chunk (cast to bf16; norms folded into oh/hb)
                aggT = atpool.tile([128, CHUNK_WINS * WIN], BF16, tag="aggT")
                nc.vector.tensor_copy(aggT[:, :cw], pagg[:, :cw])

                # h^T slab (host-pretransposed)
                hT = htpool.tile([128, CHUNK_WINS * WIN], BF16, tag="hT")
                nc.sync.dma_start(hT[:, :cw], hTd[:, col0:col0 + cw])

                # out^T = W1.T @ h^T + W2.T @ agg^T ; +bias; L2 normalize
                for bs in range(0, cw, BANK):
                    bw = min(BANK, cw - bs)
                    po = scr_ps.tile([128, BANK], F32, tag="po")
                    nc.tensor.matmul(po[:, :bw], lhsT=w1_sb[:], rhs=hT[:, bs:bs + bw],
                                     start=True, stop=False)
                    nc.tensor.matmul(po[:, :bw], lhsT=w2_sb[:], rhs=aggT[:, bs:bs + bw],
                                     start=False, stop=True)
                    y = ypool.tile([128, BANK], F32, tag="y")
                    nc.scalar.activation(y[:, :bw], po[:, :bw],
                                         mybir.ActivationFunctionType.Identity,
                                         bias=bias_sb[:])
                    z = ypool.tile([128, BANK], BF16, tag="z")
                    nc.scalar.square(z[:, :bw], y[:, :bw])
                    pr = scr_ps.tile([128, BANK], F32, tag="pr")
                    nc.tensor.matmul(pr[:, :bw], lhsT=ones_sb[:], rhs=z[:, :bw],
                                     start=True, stop=True)
                    lg = ypool.tile([128, BANK], F32, tag="lg")
                    nc.scalar.activation(lg[:, :bw], pr[:, :bw],
                                         mybir.ActivationFunctionType.Ln)
                    rs = ypool.tile([128, BANK], F32, tag="rs")
                    nc.scalar.activation(rs[:, :bw], lg[:, :bw],
                                         mybir.ActivationFunctionType.Exp,
                                         scale=-0.5)
                    of = ypool.tile([128, BANK], F32, tag="of")
                    nc.vector.tensor_tensor(out=of[:, :bw], in0=y[:, :bw],
                                            in1=rs[:, :bw], op=mybir.AluOpType.mult)
                    nc.sync.dma_start(out_p[:, col0 + bs:col0 + bs + bw], of[:, :bw])

    nc.finalize()
    if split_waits:
        _split_excess_waits(nc)
    return nc


def _run(h, weight, bias, src, dst, n_nodes, npc, cores, trace=False):
    sched = _preprocess(src, dst, n_nodes, npc, cores)
    nc = _build_program(sched, n_nodes, npc)

    padn = sched["n_wins"] * WIN
    h = np.asarray(h, dtype=np.float32)
    # gather table pre-scaled by rsqrt(deg_src)
    hb = (h * sched["nrm"][:, None].astype(np.float32)).astype(ml_dtypes.bfloat16)
    wt = np.asarray(weight, dtype=np.float32).astype(ml_dtypes.bfloat16)
    bias_c = np.ascontiguousarray(np.asarray(bias, dtype=np.float32).reshape(HID, 1))

    in_maps = []
    for c in range(cores):
        pc = sched["per_core"][c]
        hTd = np.zeros((D, padn), dtype=ml_dtypes.bfloat16)
        hTd[:, :npc] = h[c * npc:(c + 1) * npc].T.astype(ml_dtypes.bfloat16)
        in_maps.append(dict(
            hb=hb, hTd=np.ascontiguousarray(hTd),
            gidx16=pc["gidx16"], offs=pc["offs"], nrmd=pc["nrmd"],
            wt=wt, bias_c=bias_c,
        ))

    res = run_bass_kernel_spmd(nc, in_maps, core_ids=list(range(cores)), trace=trace)
    out = np.empty((cores * npc, HID), dtype=np.float32)
    for c in range(cores):
        out[c * npc:(c + 1) * npc] = res.results[c]["out"][:, :npc].T
    return out, res


def kernel(h, weight, bias, src, dst):
    out, _ = _run(h, weight, bias, src, dst, N_NODES, N_NODES // CORES, CORES)
    return out


# revision 16
# speedup vs baseline: 1.0183x; 1.0183x over previous
"""TAGConv-style GNN encoder (degree-normalized edge aggregation + linear +
L2 row-normalize) on 8 Trainium2 NeuronCores.

Strategy (dst-sharded, fully data-parallel — no collectives):
  - Nodes sharded by destination: core c owns dst rows [c*NPC, (c+1)*NPC).
  - Host-side metadata: edges (with multiplicity — no dedup) are laid out
    into 128-edge tiles grouped by (256-wide dst window, src-chunk of 25000
    rows). The tile schedule is shared across cores (padded to the
    per-region max) so one SPMD program serves all 8.
  - Gather: the h table is pre-scaled by rsqrt(deg_src) on host (bf16).
    Per region, one big gpsimd dma_gather in PREPARE_ONLY mode writes SWDGE
    descriptors; trigger_dma fires them. 4 SWDGE queues + a 4096-descriptor
    ring let descriptor generation overlap the DMA transfers, so the DMA
    engines (not the gpsimd ucode) are the limiter.
  - Scatter: DVE tensor_scalar builds per-tile one-hot segment matrices
    oh[slot, dstoff] = (iota == offs[slot]) * rsqrt(deg_dst[slot]) in one
    4x-mode instruction per tile; TensorE matmul G.T @ oh accumulates
    segment sums in PSUM across tiles (has_written semantics).
  - Tail: out^T = W1.T @ h^T + W2.T @ agg^T (+bias), L2-normalize columns
    via ones-matmul partition reduction + scalar-engine Rsqrt. h^T comes
    pre-transposed from the host. Output is written transposed
    [128, NPC_padded]; the host transposes/concatenates shards.
"""
import numpy as np
import ml_dtypes

import concourse.bass as bass
import concourse.bass_isa as bass_isa
import concourse.tile as tile
from concourse import mybir, bacc
from concourse.bass_utils import run_bass_kernel_spmd

F32 = mybir.dt.float32
BF16 = mybir.dt.bfloat16
I32 = mybir.dt.int32
I16 = mybir.dt.int16


def _patched_drain_and_barrier(self, tick_clock, wait_clock):
    """Tile's kernel-tail Drain carries one sync-wait per outstanding
    semaphore; the walrus build in this container can't encode more than one
    wait on one instruction. Emit each wait as its own wait_ge instead."""
    nc = self.nc
    probe = nc.sync.nop(nofuse=True)
    wait_clock.add_sem_waits(probe.ins, tile.ScopedClock({None: tick_clock.global_clock}))
    si = probe.ins.sync_info
    waits = list(si.on_wait) if si is not None else []
    if len(waits) > 1:
        si.on_wait.clear()
        sem_by_num = {h.num: h for h in self.sems.allocated().values()}
        for w in waits:
            nc.sync.wait_ge(sem_by_num[w.id], w.wait_value)
    nc.sync.drain()
    nc.all_engine_barrier()
    popped = nc._tile_sem_poison_stack.pop()
    assert popped is self._sem_poison
    nc.clear_and_free_semaphores(list(self.sems.allocated().values()))
    nc.all_engine_barrier()


tile.TileContext._drain_and_barrier = _patched_drain_and_barrier

# this walrus build encodes at most this many sync waits on one instruction
MAX_WAITS = 1


def _split_excess_waits(nc, max_waits=MAX_WAITS):
    """Hoist sync waits beyond the per-instruction ISA budget onto NoOps
    inserted just before the instruction (same engine queue, so ordering
    semantics are identical). Must run AFTER Bacc.compile (its nop-fusion
    passes would re-merge the waits)."""
    for f in nc.m.functions:
        for b in f.blocks:
            ins_list = b.instructions
            out_list = []
            changed = False
            for ins in ins_list:
                si = ins.sync_info
                waits = list(si.on_wait) if si is not None else []
                if len(waits) > max_waits:
                    excess, keep = waits[:-max_waits], waits[-max_waits:]
                    for j in range(0, len(excess), max_waits):
                        nop = mybir.InstNoOp(
                            name=nc.get_next_instruction_name(), ins=[], outs=[])
                        nop.engine = ins.engine
                        nop.sync_info = mybir.SyncInfo(
                            on_wait=excess[j:j + max_waits], on_update=[])
                        out_list.append(nop)
                    ins.sync_info = mybir.SyncInfo(
                        on_wait=keep, on_update=list(si.on_update))
                    changed = True
                out_list.append(ins)
            if changed:
                b.instructions = out_list


# Problem constants (hardcoded: harness contract)
N_NODES = 100000
D = 128
HID = 128
CORES = 8

# Kernel tuning
WIN = 256         # dst window width = segment-matmul N
TILE = 128        # edge slots per tile (= matmul K)
BANK = 512        # PSUM bank width in f32 cols
CHUNK_WINS = 6    # windows per PSUM chunk (6*256 = 1536 cols = 3 banks)
GXMAX = 8         # max tiles per dma_gather
SCH = 4           # src chunks (int16 gather indices => table <= 32767 rows)
NQ = 4            # SWDGE queues
SCRATCH = 16384   # dynamic DMA scratch bytes/partition
PREP = True       # prepare_only + trigger_dma (False: blocking dma_gather)
OFFS_PAD = 512.0  # one-hot match value for pad slots (never matches iota)



def _preprocess(src, dst, n_nodes, npc, cores):
    """Host-side edge partitioning (integer index metadata only)."""
    assert n_nodes % SCH == 0
    cn = n_nodes // SCH
    assert cn < 32768, "src-chunk must fit int16 gather indices"
    src = np.asarray(src).astype(np.int64)
    dst = np.asarray(dst).astype(np.int64)
    E = len(src)
    deg = np.bincount(dst, minlength=n_nodes)
    nrm = 1.0 / np.sqrt(np.maximum(deg, 1).astype(np.float64))

    core_of = dst // npc
    ldst = dst - core_of * npc
    win = ldst // WIN
    kch = src // cn
    n_wins = (npc + WIN - 1) // WIN
    n_regions = n_wins * SCH
    region = win * SCH + kch  # (w, k) region id within a core

    # program emission order of regions: psum-chunk major, then k, then w
    order_regions = []
    for p0 in range(0, n_wins, CHUNK_WINS):
        p1 = min(n_wins, p0 + CHUNK_WINS)
        for k in range(SCH):
            for w in range(p0, p1):
                order_regions.append(w * SCH + k)
    order_regions = np.array(order_regions, np.int64)
    region_pos = np.empty(n_regions, np.int64)
    region_pos[order_regions] = np.arange(n_regions)

    # per-core edge counts per region; shared tile schedule = per-region max
    cnt = np.zeros((cores, n_regions), np.int64)
    for c in range(cores):
        m = core_of == c
        cnt[c] = np.bincount(region[m], minlength=n_regions)
    tiles_r = -(-cnt.max(axis=0) // TILE)  # ceil
    # every window must write its PSUM cols at least once
    win_tiles = tiles_r.reshape(n_wins, SCH).sum(axis=1)
    for w in np.nonzero(win_tiles == 0)[0]:
        tiles_r[w * SCH] = 1

    T_sorted = tiles_r[order_regions]
    tile_base_sorted = np.zeros(n_regions + 1, np.int64)
    np.cumsum(T_sorted, out=tile_base_sorted[1:])
    n_tiles = int(tile_base_sorted[-1])
    n_slots = n_tiles * TILE
    slot_base_pos = tile_base_sorted[:-1] * TILE  # by emission pos

    # slot assignment: sort edges by (core, region emission pos, src)
    rpos = region_pos[region]
    glob = np.lexsort((src, rpos, core_of))
    cs, rs = core_of[glob], rpos[glob]
    runkey = cs * n_regions + rs
    starts = np.searchsorted(runkey, np.arange(cores * n_regions))
    rank = np.arange(E) - starts[runkey]
    slot = slot_base_pos[rs] + rank

    core_bounds = np.searchsorted(cs, np.arange(cores + 1))

    per_core = []
    for c in range(cores):
        s, e = core_bounds[c], core_bounds[c + 1]
        sl = slot[s:e]
        g = glob[s:e]
        gidx = np.zeros(n_slots, np.int16)        # pads gather row 0 of chunk
        offs = np.full(n_slots, OFFS_PAD, np.float32)
        nrmd = np.zeros(n_slots, np.float32)
        gidx[sl] = (src[g] - kch[g] * cn).astype(np.int16)
        offs[sl] = (ldst[g] - win[g] * WIN).astype(np.float32)
        nrmd[sl] = nrm[dst[g]].astype(np.float32)

        # [n_slots] -> [128, n_tiles]: slot j of tile t at [j, t]
        def t_(a, dt):
            return np.ascontiguousarray(a.reshape(n_tiles, TILE).T.astype(dt))

        # int16 idx wrap for dma_gather: within-instruction idx i at
        # [i % 16, i // 16], replicated across the 8 16-partition groups.
        # Instruction = run of whole tiles, so per-tile 8-col blocks suffice.
        a = gidx.reshape(n_tiles, 8, 16)          # [t, i//16, i%16]
        wrapped = np.transpose(a, (2, 0, 1)).reshape(16, n_tiles * 8)
        gidx16 = np.ascontiguousarray(np.tile(wrapped, (8, 1)))  # [128, 8*NT]

        per_core.append(dict(
            gidx16=gidx16,
            offs=t_(offs, np.float32),
            nrmd=t_(nrmd, np.float32),
        ))

    # gather batches: consecutive same-k regions packed up to GXMAX tiles,
    # never crossing a psum-chunk boundary
    groups = []  # (k, t_start, t_end)
    n_chunks = -(-n_wins // CHUNK_WINS)
    pos = 0
    t_acc = 0
    for p0 in range(0, n_wins, CHUNK_WINS):
        p1 = min(n_wins, p0 + CHUNK_WINS)
        for k in range(SCH):
            run = []  # tile counts of this (chunk, k) run of regions
            for w in range(p0, p1):
                run.append(int(tiles_r[w * SCH + k]))
            total = sum(run)
            # split [t_acc, t_acc+total) into <=GXMAX batches
            a = 0
            while a < total:
                b = min(total, a + GXMAX)
                groups.append((k, t_acc + a, t_acc + b))
                a = b
            t_acc += total
            pos += len(run)
    assert t_acc == n_tiles

    # window of each tile (for matmul column/bank mapping)
    win_of_tile = np.empty(n_tiles, np.int64)
    t = 0
    for p0 in range(0, n_wins, CHUNK_WINS):
        p1 = min(n_wins, p0 + CHUNK_WINS)
        for k in range(SCH):
            for w in range(p0, p1):
                nt = int(tiles_r[w * SCH + k])
                win_of_tile[t:t + nt] = w
                t += nt

    return dict(
        groups=groups,
        n_wins=n_wins,
        n_tiles=n_tiles,
        win_of_tile=win_of_tile,
        nrm=nrm,
        per_core=per_core,
    )


def _build_program(sched, n_nodes, npc, split_waits=True):
    """Build the single SPMD Bass/Tile program (identical for all cores)."""
    n_wins = sched["n_wins"]
    n_tiles = sched["n_tiles"]
    win_of_tile = sched["win_of_tile"]
    groups = sched["groups"]
    cn = n_nodes // SCH
    padn = n_wins * WIN            # padded local dst count (cols of out^T)
    n_chunks = -(-n_wins // CHUNK_WINS)

    nc = bacc.Bacc("TRN2", target_bir_lowering=False,
                   num_swdge_queues=NQ, dynamic_dma_scratch_size=SCRATCH)
    hb = nc.declare_dram_parameter("hb", [n_nodes, D], BF16, isOutput=False)
    hTd = nc.declare_dram_parameter("hTd", [D, padn], BF16, isOutput=False)
    gidx_p = nc.declare_dram_parameter("gidx16", [TILE, 8 * n_tiles], I16, isOutput=False)
    offs_p = nc.declare_dram_parameter("offs", [TILE, n_tiles], F32, isOutput=False)
    nrmd_p = nc.declare_dram_parameter("nrmd", [TILE, n_tiles], F32, isOutput=False)
    wt_p = nc.declare_dram_parameter("wt", [2 * D, HID], BF16, isOutput=False)
    bias_p = nc.declare_dram_parameter("bias_c", [HID, 1], F32, isOutput=False)
    out_p = nc.declare_dram_parameter("out", [HID, padn], F32, isOutput=True)

    # tiles grouped per psum chunk
    chunk_of_tile = win_of_tile // CHUNK_WINS

    with tile.TileContext(nc) as tc:
        with (
            tc.tile_pool(name="const", bufs=1) as const,
            tc.tile_pool(name="g", bufs=5) as gpool,
            tc.tile_pool(name="oh", bufs=4) as ohpool,
            tc.tile_pool(name="ht", bufs=2) as htpool,
            tc.tile_pool(name="at", bufs=2) as atpool,
            tc.tile_pool(name="y", bufs=6) as ypool,
            tc.tile_pool(name="aggps", bufs=1, space="PSUM") as agg_ps,
            tc.tile_pool(name="scrps", bufs=2, space="PSUM") as scr_ps,
        ):
            # ---- constants / metadata ----
            gidx_sb = const.tile([TILE, 8 * n_tiles], I16)
            nc.sync.dma_start(gidx_sb[:], gidx_p[:])
            offs_sb = const.tile([TILE, n_tiles], F32)
            nc.sync.dma_start(offs_sb[:], offs_p[:])
            nrmd_sb = const.tile([TILE, n_tiles], F32)
            nc.sync.dma_start(nrmd_sb[:], nrmd_p[:])

            w1_sb = const.tile([D, HID], BF16)
            nc.sync.dma_start(w1_sb[:], wt_p[0:D, :])
            w2_sb = const.tile([D, HID], BF16)
            nc.sync.dma_start(w2_sb[:], wt_p[D:2 * D, :])
            bias_sb = const.tile([HID, 1], F32)
            nc.sync.dma_start(bias_sb[:], bias_p[:])
            ones_sb = const.tile([128, 128], BF16)
            nc.vector.memset(ones_sb[:], 1.0)
            iota_i = const.tile([128, WIN], I32)
            nc.gpsimd.iota(iota_i[:], pattern=[[1, WIN]], base=0, channel_multiplier=0)
            iota_b = const.tile([128, WIN], BF16)
            nc.vector.tensor_copy(iota_b[:], iota_i[:])

            # shared num_idxs registers for dma_gather (one per distinct size)
            ni_regs = {}

            def ni_reg(n):
                if n not in ni_regs:
                    r = nc.gpsimd.alloc_register()
                    nc.gpsimd.reg_mov(r, n)
                    ni_regs[n] = r
                return ni_regs[n]

            dma_sems = [nc.alloc_semaphore(f"gdma{q}") for q in range(NQ)]

            # per-chunk batch lists
            batches_of_chunk = [[] for _ in range(n_chunks)]
            for gi, (k, ta, tb) in enumerate(groups):
                ch = int(chunk_of_tile[ta])
                assert int(chunk_of_tile[tb - 1]) == ch
                batches_of_chunk[ch].append((k, ta, tb))

            qrr = 0  # round-robin queue cursor
            qcount = [0] * NQ  # completed-DMA wait targets per queue

            # ---- main loop over dst chunks ----
            for ch in range(n_chunks):
                w0 = ch * CHUNK_WINS
                w1 = min(n_wins, w0 + CHUNK_WINS)
                cw = (w1 - w0) * WIN
                col0 = w0 * WIN

                tlist = [t for (k, ta, tb) in batches_of_chunk[ch]
                         for t in range(ta, tb)]
                # first/last program-order touch per psum bank in this chunk
                first_of_bank, last_of_bank = {}, {}
                for t in tlist:
                    bk = (int(win_of_tile[t]) - w0) * WIN // BANK
                    first_of_bank.setdefault(bk, t)
                    last_of_bank[bk] = t

                pagg = agg_ps.tile([128, CHUNK_WINS * WIN], F32, tag="pagg")

                for (k, ta, tb) in batches_of_chunk[ch]:
                    gt = tb - ta
                    G = gpool.tile([128, GXMAX, D], BF16, tag="G")
                    q = qrr % NQ
                    qrr += 1
                    qcount[q] += 1
                    gwait = 16 * qcount[q]
                    if PREP:
                        nc.gpsimd.dma_gather(
                            out_ap=G[:, :gt, :],
                            in_ap=hb[k * cn:(k + 1) * cn, :],
                            idxs_ap=gidx_sb[:, 8 * ta:8 * tb],
                            num_idxs=TILE * gt,
                            num_idxs_reg=ni_reg(TILE * gt),
                            elem_size=D,
                            prepare_only=True,
                            sem=dma_sems[q],
                            queue_num=q,
                        )
                        nc.gpsimd.trigger_dma(count=None, queue_num=q)
                    else:
                        nc.gpsimd.dma_gather(
                            out_ap=G[:, :gt, :],
                            in_ap=hb[k * cn:(k + 1) * cn, :],
                            idxs_ap=gidx_sb[:, 8 * ta:8 * tb],
                            num_idxs=TILE * gt,
                            num_idxs_reg=ni_reg(TILE * gt),
                            elem_size=D,
                            queue_num=q,
                        )

                    oh = ohpool.tile([128, GXMAX, WIN], BF16, tag="oh")
                    for x in range(gt):
                        t = ta + x
                        nc.vector.tensor_scalar(
                            out=oh[:, x, :],
                            in0=iota_b[:],
                            scalar1=offs_sb[:, t:t + 1],
                            scalar2=nrmd_sb[:, t:t + 1],
                            op0=mybir.AluOpType.is_equal,
                            op1=mybir.AluOpType.mult,
                        )
                    if PREP:
                        # Tile's DMASW lane sems are pre-bumped for SWDGE
                        # preps; enforce the gather-DMA RAW edge explicitly.
                        nc.tensor.wait_ge(dma_sems[q], gwait)
                    for x in range(gt):
                        t = ta + x
                        col = (int(win_of_tile[t]) - w0) * WIN
                        bk = col // BANK
                        nc.tensor.matmul(
                            pagg[:, col:col + WIN],
                            lhsT=G[:, x, :],
                            rhs=oh[:, x, :],
                            start=(first_of_bank[bk] == t),
                            stop=(last_of_bank[bk] == t),
                            skip_group_check=True,
                        )

                # evacuate agg chunk (cast to bf16; norms folded into oh/hb)
                aggT = atpool.tile([128, CHUNK_WINS * WIN], BF16, tag="aggT")
                nc.vector.tensor_copy(aggT[:, :cw], pagg[:, :cw])

                # h^T slab (host-pretransposed)
                hT = htpool.tile([128, CHUNK_WINS * WIN], BF16, tag="hT")
                nc.sync.dma_start(hT[:, :cw], hTd[:, col0:col0 + cw])

                # out^T = W1.T @ h^T + W2.T @ agg^T ; +bias; L2 normalize
                for bs in range(0, cw, BANK):
                    bw = min(BANK, cw - bs)
                    po = scr_ps.tile([128, BANK], F32, tag="po")
                    nc.tensor.matmul(po[:, :bw], lhsT=w1_sb[:], rhs=hT[:, bs:bs + bw],
                                     start=True, stop=False)
                    nc.tensor.matmul(po[:, :bw], lhsT=w2_sb[:], rhs=aggT[:, bs:bs + bw],
                                     start=False, stop=True)
                    y = ypool.tile([128, BANK], F32, tag="y")
                    nc.scalar.activation(y[:, :bw], po[:, :bw],
                                         mybir.ActivationFunctionType.Identity,
                                         bias=bias_sb[:])
                    z = ypool.tile([128, BANK], BF16, tag="z")
                    nc.scalar.square(z[:, :bw], y[:, :bw])
                    pr = scr_ps.tile([128, BANK], F32, tag="pr")
                    nc.tensor.matmul(pr[:, :bw], lhsT=ones_sb[:], rhs=z[:, :bw],
                                     start=True, stop=True)
                    lg = ypool.tile([128, BANK], F32, tag="lg")
                    nc.scalar.activation(lg[:, :bw], pr[:, :bw],
                                         mybir.ActivationFunctionType.Ln)
                    rs = ypool.tile([128, BANK], F32, tag="rs")
                    nc.scalar.activation(rs[:, :bw], lg[:, :bw],
                                         mybir.ActivationFunctionType.Exp,
                                         scale=-0.5)
                    of = ypool.tile([128, BANK], F32, tag="of")
                    nc.vector.tensor_tensor(out=of[:, :bw], in0=y[:, :bw],
                                            in1=rs[:, :bw], op=mybir.AluOpType.mult)
                    nc.sync.dma_start(out_p[:, col0 + bs:col0 + bs + bw], of[:, :bw])

    nc.finalize()
    if split_waits:
        _split_excess_waits(nc)
    return nc


def _run(h, weight, bias, src, dst, n_nodes, npc, cores, trace=False):
    sched = _preprocess(src, dst, n_nodes, npc, cores)
    nc = _build_program(sched, n_nodes, npc)

    padn = sched["n_wins"] * WIN
    h = np.asarray(h, dtype=np.float32)
    # gather table pre-scaled by rsqrt(deg_src)
    hb = (h * sched["nrm"][:, None].astype(np.float32)).astype(ml_dtypes.bfloat16)
    wt = np.asarray(weight, dtype=np.float32).astype(ml_dtypes.bfloat16)
    bias_c = np.ascontiguousarray(np.asarray(bias, dtype=np.float32).reshape(HID, 1))

    in_maps = []
    for c in range(cores):
        pc = sched["per_core"][c]
        hTd = np.zeros((D, padn), dtype=ml_dtypes.bfloat16)
        hTd[:, :npc] = h[c * npc:(c + 1) * npc].T.astype(ml_dtypes.bfloat16)
        in_maps.append(dict(
            hb=hb, hTd=np.ascontiguousarray(hTd),
            gidx16=pc["gidx16"], offs=pc["offs"], nrmd=pc["nrmd"],
            wt=wt, bias_c=bias_c,
        ))

    res = run_bass_kernel_spmd(nc, in_maps, core_ids=list(range(cores)), trace=trace)
    out = np.empty((cores * npc, HID), dtype=np.float32)
    for c in range(cores):
        out[c * npc:(c + 1) * npc] = res.results[c]["out"][:, :npc].T
    return out, res


def kernel(h, weight, bias, src, dst):
    out, _ = _run(h, weight, bias, src, dst, N_NODES, N_NODES // CORES, CORES)
    return out


# revision 17
# speedup vs baseline: 1.7061x; 1.6754x over previous
"""TAGConv-style GNN encoder (degree-normalized edge aggregation + linear +
L2 row-normalize) on 8 Trainium2 NeuronCores.

Strategy (dst-sharded, fully data-parallel — no collectives):
  - Nodes sharded by destination: core c owns dst rows [c*NPC, (c+1)*NPC).
  - Host-side metadata: edges (with multiplicity — no dedup) are laid out
    into 128-edge tiles grouped by (256-wide dst window, src-chunk of 25000
    rows). The tile schedule is shared across cores (padded to the
    per-region max) so one SPMD program serves all 8.
  - Gather: the h table is pre-scaled by rsqrt(deg_src) on host (bf16).
    Per region, one big gpsimd dma_gather in PREPARE_ONLY mode writes SWDGE
    descriptors; trigger_dma fires them. 4 SWDGE queues + a 4096-descriptor
    ring let descriptor generation overlap the DMA transfers, so the DMA
    engines (not the gpsimd ucode) are the limiter.
  - Scatter: DVE tensor_scalar builds per-tile one-hot segment matrices
    oh[slot, dstoff] = (iota == offs[slot]) * rsqrt(deg_dst[slot]) in one
    4x-mode instruction per tile; TensorE matmul G.T @ oh accumulates
    segment sums in PSUM across tiles (has_written semantics).
  - Tail: out^T = W1.T @ h^T + W2.T @ agg^T (+bias), L2-normalize columns
    via ones-matmul partition reduction + scalar-engine Rsqrt. h^T comes
    pre-transposed from the host. Output is written transposed
    [128, NPC_padded]; the host transposes/concatenates shards.
"""
import numpy as np
import ml_dtypes

import concourse.bass as bass
import concourse.bass_isa as bass_isa
import concourse.tile as tile
from concourse import mybir, bacc
from concourse.bass_utils import run_bass_kernel_spmd

F32 = mybir.dt.float32
BF16 = mybir.dt.bfloat16
I32 = mybir.dt.int32
I16 = mybir.dt.int16


def _patched_drain_and_barrier(self, tick_clock, wait_clock):
    """Tile's kernel-tail Drain carries one sync-wait per outstanding
    semaphore; the walrus build in this container can't encode more than one
    wait on one instruction. Emit each wait as its own wait_ge instead."""
    nc = self.nc
    probe = nc.sync.nop(nofuse=True)
    wait_clock.add_sem_waits(probe.ins, tile.ScopedClock({None: tick_clock.global_clock}))
    si = probe.ins.sync_info
    waits = list(si.on_wait) if si is not None else []
    if len(waits) > 1:
        si.on_wait.clear()
        sem_by_num = {h.num: h for h in self.sems.allocated().values()}
        for w in waits:
            nc.sync.wait_ge(sem_by_num[w.id], w.wait_value)
    nc.sync.drain()
    nc.all_engine_barrier()
    popped = nc._tile_sem_poison_stack.pop()
    assert popped is self._sem_poison
    nc.clear_and_free_semaphores(list(self.sems.allocated().values()))
    nc.all_engine_barrier()


tile.TileContext._drain_and_barrier = _patched_drain_and_barrier

# this walrus build encodes at most this many sync waits on one instruction
MAX_WAITS = 1


def _split_excess_waits(nc, max_waits=MAX_WAITS):
    """Hoist sync waits beyond the per-instruction ISA budget onto NoOps
    inserted just before the instruction (same engine queue, so ordering
    semantics are identical). Must run AFTER Bacc.compile (its nop-fusion
    passes would re-merge the waits)."""
    for f in nc.m.functions:
        for b in f.blocks:
            ins_list = b.instructions
            out_list = []
            changed = False
            for ins in ins_list:
                si = ins.sync_info
                waits = list(si.on_wait) if si is not None else []
                if len(waits) > max_waits:
                    excess, keep = waits[:-max_waits], waits[-max_waits:]
                    for j in range(0, len(excess), max_waits):
                        nop = mybir.InstNoOp(
                            name=nc.get_next_instruction_name(), ins=[], outs=[])
                        nop.engine = ins.engine
                        nop.sync_info = mybir.SyncInfo(
                            on_wait=excess[j:j + max_waits], on_update=[])
                        out_list.append(nop)
                    ins.sync_info = mybir.SyncInfo(
                        on_wait=keep, on_update=list(si.on_update))
                    changed = True
                out_list.append(ins)
            if changed:
                b.instructions = out_list


# Problem constants (hardcoded: harness contract)
N_NODES = 100000
D = 128
HID = 128
CORES = 8

# Kernel tuning
WIN = 256         # dst window width = segment-matmul N
TILE = 128        # edge slots per tile (= matmul K)
BANK = 512        # PSUM bank width in f32 cols
CHUNK_WINS = 6    # windows per PSUM chunk (6*256 = 1536 cols = 3 banks)
GXMAX = 8         # max tiles per dma_gather
SCH = 4           # src chunks (int16 gather indices => table <= 32767 rows)
NQ = 4            # SWDGE queues
SCRATCH = 16384   # dynamic DMA scratch bytes/partition
PREP = False      # prepare_only + trigger_dma (False: blocking dma_gather)
OFFS_PAD = 512.0  # one-hot match value for pad slots (never matches iota)



def _preprocess(src, dst, n_nodes, npc, cores):
    """Host-side edge partitioning (integer index metadata only)."""
    assert n_nodes % SCH == 0
    cn = n_nodes // SCH
    assert cn < 32768, "src-chunk must fit int16 gather indices"
    src = np.asarray(src).astype(np.int64)
    dst = np.asarray(dst).astype(np.int64)
    E = len(src)
    deg = np.bincount(dst, minlength=n_nodes)
    nrm = 1.0 / np.sqrt(np.maximum(deg, 1).astype(np.float64))

    core_of = dst // npc
    ldst = dst - core_of * npc
    win = ldst // WIN
    kch = src // cn
    n_wins = (npc + WIN - 1) // WIN
    n_regions = n_wins * SCH
    region = win * SCH + kch  # (w, k) region id within a core

    # program emission order of regions: psum-chunk major, then k, then w
    order_regions = []
    for p0 in range(0, n_wins, CHUNK_WINS):
        p1 = min(n_wins, p0 + CHUNK_WINS)
        for k in range(SCH):
            for w in range(p0, p1):
                order_regions.append(w * SCH + k)
    order_regions = np.array(order_regions, np.int64)
    region_pos = np.empty(n_regions, np.int64)
    region_pos[order_regions] = np.arange(n_regions)

    # per-core edge counts per region; shared tile schedule = per-region max
    cnt = np.zeros((cores, n_regions), np.int64)
    for c in range(cores):
        m = core_of == c
        cnt[c] = np.bincount(region[m], minlength=n_regions)
    tiles_r = -(-cnt.max(axis=0) // TILE)  # ceil
    # every window must write its PSUM cols at least once
    win_tiles = tiles_r.reshape(n_wins, SCH).sum(axis=1)
    for w in np.nonzero(win_tiles == 0)[0]:
        tiles_r[w * SCH] = 1

    T_sorted = tiles_r[order_regions]
    tile_base_sorted = np.zeros(n_regions + 1, np.int64)
    np.cumsum(T_sorted, out=tile_base_sorted[1:])
    n_tiles = int(tile_base_sorted[-1])
    n_slots = n_tiles * TILE
    slot_base_pos = tile_base_sorted[:-1] * TILE  # by emission pos

    # slot assignment: sort edges by (core, region emission pos, src)
    rpos = region_pos[region]
    glob = np.lexsort((src, rpos, core_of))
    cs, rs = core_of[glob], rpos[glob]
    runkey = cs * n_regions + rs
    starts = np.searchsorted(runkey, np.arange(cores * n_regions))
    rank = np.arange(E) - starts[runkey]
    slot = slot_base_pos[rs] + rank

    core_bounds = np.searchsorted(cs, np.arange(cores + 1))

    per_core = []
    for c in range(cores):
        s, e = core_bounds[c], core_bounds[c + 1]
        sl = slot[s:e]
        g = glob[s:e]
        gidx = np.zeros(n_slots, np.int16)        # pads gather row 0 of chunk
        offs = np.full(n_slots, OFFS_PAD, np.float32)
        nrmd = np.zeros(n_slots, np.float32)
        gidx[sl] = (src[g] - kch[g] * cn).astype(np.int16)
        offs[sl] = (ldst[g] - win[g] * WIN).astype(np.float32)
        nrmd[sl] = nrm[dst[g]].astype(np.float32)

        # [n_slots] -> [128, n_tiles]: slot j of tile t at [j, t]
        def t_(a, dt):
            return np.ascontiguousarray(a.reshape(n_tiles, TILE).T.astype(dt))

        # int16 idx wrap for dma_gather: within-instruction idx i at
        # [i % 16, i // 16], replicated across the 8 16-partition groups.
        # Instruction = run of whole tiles, so per-tile 8-col blocks suffice.
        a = gidx.reshape(n_tiles, 8, 16)          # [t, i//16, i%16]
        wrapped = np.transpose(a, (2, 0, 1)).reshape(16, n_tiles * 8)
        gidx16 = np.ascontiguousarray(np.tile(wrapped, (8, 1)))  # [128, 8*NT]

        per_core.append(dict(
            gidx16=gidx16,
            offs=t_(offs, np.float32),
            nrmd=t_(nrmd, np.float32),
        ))

    # gather batches: consecutive same-k regions packed up to GXMAX tiles,
    # never crossing a psum-chunk boundary
    groups = []  # (k, t_start, t_end)
    n_chunks = -(-n_wins // CHUNK_WINS)
    pos = 0
    t_acc = 0
    for p0 in range(0, n_wins, CHUNK_WINS):
        p1 = min(n_wins, p0 + CHUNK_WINS)
        for k in range(SCH):
            run = []  # tile counts of this (chunk, k) run of regions
            for w in range(p0, p1):
                run.append(int(tiles_r[w * SCH + k]))
            total = sum(run)
            # split [t_acc, t_acc+total) into <=GXMAX batches
            a = 0
            while a < total:
                b = min(total, a + GXMAX)
                groups.append((k, t_acc + a, t_acc + b))
                a = b
            t_acc += total
            pos += len(run)
    assert t_acc == n_tiles

    # window of each tile (for matmul column/bank mapping)
    win_of_tile = np.empty(n_tiles, np.int64)
    t = 0
    for p0 in range(0, n_wins, CHUNK_WINS):
        p1 = min(n_wins, p0 + CHUNK_WINS)
        for k in range(SCH):
            for w in range(p0, p1):
                nt = int(tiles_r[w * SCH + k])
                win_of_tile[t:t + nt] = w
                t += nt

    return dict(
        groups=groups,
        n_wins=n_wins,
        n_tiles=n_tiles,
        win_of_tile=win_of_tile,
        nrm=nrm,
        per_core=per_core,
    )


def _build_program(sched, n_nodes, npc, split_waits=True):
    """Build the single SPMD Bass/Tile program (identical for all cores)."""
    n_wins = sched["n_wins"]
    n_tiles = sched["n_tiles"]
    win_of_tile = sched["win_of_tile"]
    groups = sched["groups"]
    cn = n_nodes // SCH
    padn = n_wins * WIN            # padded local dst count (cols of out^T)
    n_chunks = -(-n_wins // CHUNK_WINS)

    nc = bacc.Bacc("TRN2", target_bir_lowering=False,
                   num_swdge_queues=NQ, dynamic_dma_scratch_size=SCRATCH)
    hb = nc.declare_dram_parameter("hb", [n_nodes, D], BF16, isOutput=False)
    hTd = nc.declare_dram_parameter("hTd", [D, padn], BF16, isOutput=False)
    gidx_p = nc.declare_dram_parameter("gidx16", [TILE, 8 * n_tiles], I16, isOutput=False)
    offs_p = nc.declare_dram_parameter("offs", [TILE, n_tiles], F32, isOutput=False)
    nrmd_p = nc.declare_dram_parameter("nrmd", [TILE, n_tiles], F32, isOutput=False)
    wt_p = nc.declare_dram_parameter("wt", [2 * D, HID], BF16, isOutput=False)
    bias_p = nc.declare_dram_parameter("bias_c", [HID, 1], F32, isOutput=False)
    out_p = nc.declare_dram_parameter("out", [HID, padn], F32, isOutput=True)

    # tiles grouped per psum chunk
    chunk_of_tile = win_of_tile // CHUNK_WINS

    with tile.TileContext(nc) as tc:
        with (
            tc.tile_pool(name="const", bufs=1) as const,
            tc.tile_pool(name="g", bufs=5) as gpool,
            tc.tile_pool(name="oh", bufs=4) as ohpool,
            tc.tile_pool(name="ht", bufs=2) as htpool,
            tc.tile_pool(name="at", bufs=2) as atpool,
            tc.tile_pool(name="y", bufs=6) as ypool,
            tc.tile_pool(name="aggps", bufs=1, space="PSUM") as agg_ps,
            tc.tile_pool(name="scrps", bufs=2, space="PSUM") as scr_ps,
        ):
            # ---- constants / metadata ----
            gidx_sb = const.tile([TILE, 8 * n_tiles], I16)
            nc.sync.dma_start(gidx_sb[:], gidx_p[:])
            offs_sb = const.tile([TILE, n_tiles], F32)
            nc.sync.dma_start(offs_sb[:], offs_p[:])
            nrmd_sb = const.tile([TILE, n_tiles], F32)
            nc.sync.dma_start(nrmd_sb[:], nrmd_p[:])

            w1_sb = const.tile([D, HID], BF16)
            nc.sync.dma_start(w1_sb[:], wt_p[0:D, :])
            w2_sb = const.tile([D, HID], BF16)
            nc.sync.dma_start(w2_sb[:], wt_p[D:2 * D, :])
            bias_sb = const.tile([HID, 1], F32)
            nc.sync.dma_start(bias_sb[:], bias_p[:])
            ones_sb = const.tile([128, 128], BF16)
            nc.vector.memset(ones_sb[:], 1.0)
            iota_i = const.tile([128, WIN], I32)
            nc.gpsimd.iota(iota_i[:], pattern=[[1, WIN]], base=0, channel_multiplier=0)
            iota_b = const.tile([128, WIN], BF16)
            nc.vector.tensor_copy(iota_b[:], iota_i[:])

            # shared num_idxs registers for dma_gather (one per distinct size)
            ni_regs = {}

            def ni_reg(n):
                if n not in ni_regs:
                    r = nc.gpsimd.alloc_register()
                    nc.gpsimd.reg_mov(r, n)
                    ni_regs[n] = r
                return ni_regs[n]

            dma_sems = [nc.alloc_semaphore(f"gdma{q}") for q in range(NQ)]

            # per-chunk batch lists
            batches_of_chunk = [[] for _ in range(n_chunks)]
            for gi, (k, ta, tb) in enumerate(groups):
                ch = int(chunk_of_tile[ta])
                assert int(chunk_of_tile[tb - 1]) == ch
                batches_of_chunk[ch].append((k, ta, tb))

            qrr = 0  # round-robin queue cursor
            qcount = [0] * NQ  # completed-DMA wait targets per queue

            # ---- main loop over dst chunks ----
            for ch in range(n_chunks):
                w0 = ch * CHUNK_WINS
                w1 = min(n_wins, w0 + CHUNK_WINS)
                cw = (w1 - w0) * WIN
                col0 = w0 * WIN

                tlist = [t for (k, ta, tb) in batches_of_chunk[ch]
                         for t in range(ta, tb)]
                # first/last program-order touch per psum bank in this chunk
                first_of_bank, last_of_bank = {}, {}
                for t in tlist:
                    bk = (int(win_of_tile[t]) - w0) * WIN // BANK
                    first_of_bank.setdefault(bk, t)
                    last_of_bank[bk] = t

                pagg = agg_ps.tile([128, CHUNK_WINS * WIN], F32, tag="pagg")

                for (k, ta, tb) in batches_of_chunk[ch]:
                    gt = tb - ta
                    G = gpool.tile([128, GXMAX, D], BF16, tag="G")
                    q = 1 + (qrr % (NQ - 1))
                    qrr += 1
                    qcount[q] += 1
                    gwait = 16 * qcount[q]
                    if PREP:
                        nc.gpsimd.dma_gather(
                            out_ap=G[:, :gt, :],
                            in_ap=hb[k * cn:(k + 1) * cn, :],
                            idxs_ap=gidx_sb[:, 8 * ta:8 * tb],
                            num_idxs=TILE * gt,
                            num_idxs_reg=ni_reg(TILE * gt),
                            elem_size=D,
                            prepare_only=True,
                            sem=dma_sems[q],
                            queue_num=q,
                        )
                        nc.gpsimd.trigger_dma(count=None, queue_num=q)
                    else:
                        nc.gpsimd.dma_gather(
                            out_ap=G[:, :gt, :],
                            in_ap=hb[k * cn:(k + 1) * cn, :],
                            idxs_ap=gidx_sb[:, 8 * ta:8 * tb],
                            num_idxs=TILE * gt,
                            num_idxs_reg=ni_reg(TILE * gt),
                            elem_size=D,
                            queue_num=q,
                        )

                    oh = ohpool.tile([128, GXMAX, WIN], BF16, tag="oh")
                    for x in range(gt):
                        t = ta + x
                        nc.vector.tensor_scalar(
                            out=oh[:, x, :],
                            in0=iota_b[:],
                            scalar1=offs_sb[:, t:t + 1],
                            scalar2=nrmd_sb[:, t:t + 1],
                            op0=mybir.AluOpType.is_equal,
                            op1=mybir.AluOpType.mult,
                        )
                    if PREP:
                        # Tile's DMASW lane sems are pre-bumped for SWDGE
                        # preps; enforce the gather-DMA RAW edge explicitly.
                        nc.tensor.wait_ge(dma_sems[q], gwait)
                    for x in range(gt):
                        t = ta + x
                        col = (int(win_of_tile[t]) - w0) * WIN
                        bk = col // BANK
                        nc.tensor.matmul(
                            pagg[:, col:col + WIN],
                            lhsT=G[:, x, :],
                            rhs=oh[:, x, :],
                            start=(first_of_bank[bk] == t),
                            stop=(last_of_bank[bk] == t),
                            skip_group_check=True,
                        )

                # evacuate agg chunk (cast to bf16; norms folded into oh/hb)
                aggT = atpool.tile([128, CHUNK_WINS * WIN], BF16, tag="aggT")
                nc.vector.tensor_copy(aggT[:, :cw], pagg[:, :cw])

                # h^T slab (host-pretransposed)
                hT = htpool.tile([128, CHUNK_WINS * WIN], BF16, tag="hT")
                nc.sync.dma_start(hT[:, :cw], hTd[:, col0:col0 + cw])

                # out^T = W1.T @ h^T + W2.T @ agg^T ; +bias; L2 normalize
                for bs in range(0, cw, BANK):
                    bw = min(BANK, cw - bs)
                    po = scr_ps.tile([128, BANK], F32, tag="po")
                    nc.tensor.matmul(po[:, :bw], lhsT=w1_sb[:], rhs=hT[:, bs:bs + bw],
                                     start=True, stop=False)
                    nc.tensor.matmul(po[:, :bw], lhsT=w2_sb[:], rhs=aggT[:, bs:bs + bw],
                                     start=False, stop=True)
                    y = ypool.tile([128, BANK], F32, tag="y")
                    nc.scalar.activation(y[:, :bw], po[:, :bw],
                                         mybir.ActivationFunctionType.Identity,
                                         bias=bias_sb[:])
                    z = ypool.tile([128, BANK], BF16, tag="z")
                    nc.scalar.square(z[:, :bw], y[:, :bw])
                    pr = scr_ps.tile([128, BANK], F32, tag="pr")
                    nc.tensor.matmul(pr[:, :bw], lhsT=ones_sb[:], rhs=z[:, :bw],
                                     start=True, stop=True)
                    lg = ypool.tile([128, BANK], F32, tag="lg")
                    nc.scalar.activation(lg[:, :bw], pr[:, :bw],
                                         mybir.ActivationFunctionType.Ln)
                    rs = ypool.tile([128, BANK], F32, tag="rs")
                    nc.scalar.activation(rs[:, :bw], lg[:, :bw],
                                         mybir.ActivationFunctionType.Exp,
                                         scale=-0.5)
                    of = ypool.tile([128, BANK], F32, tag="of")
                    nc.vector.tensor_tensor(out=of[:, :bw], in0=y[:, :bw],
                                            in1=rs[:, :bw], op=mybir.AluOpType.mult)
                    nc.sync.dma_start(out_p[:, col0 + bs:col0 + bs + bw], of[:, :bw])

    nc.finalize()
    if split_waits:
        _split_excess_waits(nc)
    return nc


def _run(h, weight, bias, src, dst, n_nodes, npc, cores, trace=False):
    sched = _preprocess(src, dst, n_nodes, npc, cores)
    nc = _build_program(sched, n_nodes, npc)

    padn = sched["n_wins"] * WIN
    h = np.asarray(h, dtype=np.float32)
    # gather table pre-scaled by rsqrt(deg_src)
    hb = (h * sched["nrm"][:, None].astype(np.float32)).astype(ml_dtypes.bfloat16)
    wt = np.asarray(weight, dtype=np.float32).astype(ml_dtypes.bfloat16)
    bias_c = np.ascontiguousarray(np.asarray(bias, dtype=np.float32).reshape(HID, 1))

    in_maps = []
    for c in range(cores):
        pc = sched["per_core"][c]
        hTd = np.zeros((D, padn), dtype=ml_dtypes.bfloat16)
        hTd[:, :npc] = h[c * npc:(c + 1) * npc].T.astype(ml_dtypes.bfloat16)
        in_maps.append(dict(
            hb=hb, hTd=np.ascontiguousarray(hTd),
            gidx16=pc["gidx16"], offs=pc["offs"], nrmd=pc["nrmd"],
            wt=wt, bias_c=bias_c,
        ))

    res = run_bass_kernel_spmd(nc, in_maps, core_ids=list(range(cores)), trace=trace)
    out = np.empty((cores * npc, HID), dtype=np.float32)
    for c in range(cores):
        out[c * npc:(c + 1) * npc] = res.results[c]["out"][:, :npc].T
    return out, res


def kernel(h, weight, bias, src, dst):
    out, _ = _run(h, weight, bias, src, dst, N_NODES, N_NODES // CORES, CORES)
    return out
